# revision 10
# baseline (speedup 1.0000x reference)
"""Trainium2 Bass kernel for nn_ContrastiveLoss (wav2vec2-style) — v4.

Shapes (hardcoded): B=8, C=256, T=1024, M=512 masked positions, K=100
negatives. Sharding: pure data parallel — batch row b -> NeuronCore b.

Design: ONE fp8 stream of the negatives (13.1 MB/core) in c-major layout
[HG=8][128, 2, 6400] (dim1 = the two 128-channel halves). Per masked
position m the PE runs four ldweights+matmul pairs with 100-col
stationaries:

- dots^T[k, m]: lhsT = negT[m] chunk [128c, 100k] fp8, rhs = ctx[m] col,
  PSUM-accumulated over the two c-halves.
- ssq^T[k, m]:  lhsT = sq[m] chunk (squared negatives), rhs = ones col.

The elementwise squares are split across ScalarE / VectorE / GpSimd
(fp8 -> fp8, one 2D op per plane per engine per tile, shares sized by
measured rates). The ssq matmuls run one half-group behind the dots so
the PE never stalls waiting for squares. Both PE outputs share one
[128, 256] PSUM tile per group and a rectangular copy + PE-transpose
([100, 128] -> [128, 100]) into per-group [128, 200] PSUM tiles that the
epilogue (norms, logits, exp-accum logsumexp) reads directly.
"""

import numpy as np

TEMP = 0.1
EPS = 1e-8
B, C, T = 8, 256, 1024
M = 512  # masked positions per batch row
K = 100  # negatives per masked position
P = 128  # partitions
G = M // P  # m-groups per core (4)
HG = 8  # half-groups (64 m's each)
MH = M // HG  # 64
NCOL = MH * K + P  # 6528 streamed cols per half-group (incl 128-col zero pad)
# elementwise-square split (cols of each [128, 2, NCOL] tile)
# measured rates: ACT 0.857 ns/col, DVE 1.39 (fp8 1x), Pool 2.35
ACT_SQ = 3530
DVE_SQ = 1830
POOL_SQ = NCOL - ACT_SQ - DVE_SQ  # 1168

_NC = None


def _build_nc():
    import concourse.bacc as bacc
    import concourse.tile as tile
    from concourse import masks, mybir

    f32 = mybir.dt.float32
    bf16 = mybir.dt.bfloat16
    fp8 = mybir.dt.float8e4
    Alu = mybir.AluOpType
    Act = mybir.ActivationFunctionType

    nc = bacc.Bacc(trn_type="TRN2")
    negt = nc.dram_tensor("negt", [HG, P, 2, NCOL], fp8, kind="ExternalInput")
    ctxt = nc.dram_tensor("ctxt", [P, 2, M], fp8, kind="ExternalInput")
    ctxg = nc.dram_tensor("ctxg", [M, C], bf16, kind="ExternalInput")
    posg = nc.dram_tensor("posg", [M, C], bf16, kind="ExternalInput")
    rowloss = nc.dram_tensor("rowloss", [P, G], f32, kind="ExternalOutput")

    from contextlib import ExitStack

    with tile.TileContext(nc) as tc, ExitStack() as es:
        pool_specs = dict(single=1, ntp=3, sqp=4, grp=2, pg=G, scrp=2, scp=2)
        pools = {
            n: es.enter_context(tc.tile_pool(name=n, bufs=b))
            for n, b in pool_specs.items()
        }
        single, ntp, sqp, grp, pg, scrp, scp = (
            pools[n] for n in ("single", "ntp", "sqp", "grp", "pg", "scrp", "scp")
        )
        accp = es.enter_context(tc.psum_pool(name="acc", bufs=3))
        tpp = es.enter_context(tc.psum_pool(name="tp", bufs=2))

        # Pin the one ACT table that serves Square+Ln+Exp+Copy so the
        # compiler's greedy per-function set choice can't ping-pong tables
        # (each implicit ACT_TABLE_LOAD costs 1283ns).
        from concourse.hw_specs import get_activation_tables

        _tabs = list(get_activation_tables(nc.m.arch).items())
        _need = {Act.Square, Act.Ln, Act.Exp, Act.Copy}
        _set_id = next(
            i for i, (_n, _s) in enumerate(_tabs) if _need <= _s
        )
        nc.scalar.add_instruction(
            mybir.InstLoadActFuncSet(
                name=nc.get_next_instruction_name(),
                ins=[], outs=[], act_func_set_id=_set_id,
            )
        )

        identity = single.tile([P, P], f32)
        masks.make_identity(nc, identity[:])
        ones_mov = single.tile([P, 1], fp8, name="ones")
        nc.gpsimd.memset(ones_mov[:], 1.0)
        dummy = single.tile([P, C], bf16)

        nts = {}
        nts[0] = ntp.tile([P, 2, NCOL], fp8, tag="nt", name="nt0")
        nc.sync.dma_start(
            out=nts[0][:, :, 0:ACT_SQ], in_=negt[0, :, :, 0:ACT_SQ]
        )
        nc.sync.dma_start(
            out=nts[0][:, :, ACT_SQ:NCOL], in_=negt[0, :, :, ACT_SQ:NCOL]
        )
        nts[1] = ntp.tile([P, 2, NCOL], fp8, tag="nt", name="nt1")
        nc.sync.dma_start(out=nts[1][:], in_=negt[1])
        ctxt_s = single.tile([P, 2, M], fp8, name="ctxts")
        nc.sync.dma_start(out=ctxt_s[:], in_=ctxt[:, :, :])

        css_a = single.tile([P, G], f32)
        pss_a = single.tile([P, G], f32)
        cpd_a = single.tile([P, G], f32)
        crn_a = single.tile([P, G], f32)
        prn_a = single.tile([P, G], f32)
        se_a = single.tile([P, G], f32)
        lnse_a = single.tile([P, G], f32)
        nrn_a = single.tile([P, G * K], f32)
        out_t = single.tile([P, G], f32)

        gt = {}
        for g in range(G):
            gt[g] = dict(
                logits=pg.tile([P, K + 1], f32, tag="logits", name=f"logits{g}"),
            )

        def emit_prologue():
            # ctx/pos self-norms via ACT Square+accum; ctx.pos dot on DVE;
            # emitted after the first stream tiles so it doesn't delay them
            for g in range(G):
                m0 = g * P
                ctx_t = grp.tile([P, C], bf16, tag="ctx", name=f"ctx{g}")
                pos_t = grp.tile([P, C], bf16, tag="pos", name=f"pos{g}")
                nc.sync.dma_start(out=ctx_t[:], in_=ctxg[m0 : m0 + P, :])
                nc.sync.dma_start(out=pos_t[:], in_=posg[m0 : m0 + P, :])
                nc.scalar.activation(
                    out=dummy[:], in_=ctx_t[:], func=Act.Square,
                    accum_out=css_a[:, g : g + 1],
                )
                nc.scalar.activation(
                    out=dummy[:], in_=pos_t[:], func=Act.Square,
                    accum_out=pss_a[:, g : g + 1],
                )
                nc.vector.scalar_tensor_tensor(
                    out=dummy[:], in0=ctx_t[:], scalar=1.0, in1=pos_t[:],
                    op0=Alu.mult, op1=Alu.mult, accum_out=cpd_a[:, g : g + 1],
                )
            # 1/|ctx|, 1/|pos| = exp(-0.5*ln(ss)); same ACT table as Square
            nc.scalar.activation(out=lnse_a[:], in_=css_a[:], func=Act.Ln)
            nc.scalar.activation(
                out=crn_a[:], in_=lnse_a[:], func=Act.Exp, scale=-0.5
            )
            nc.scalar.activation(out=lnse_a[:], in_=pss_a[:], func=Act.Ln)
            nc.scalar.activation(
                out=prn_a[:], in_=lnse_a[:], func=Act.Exp, scale=-0.5
            )

        # ---- main stream, ssq matmuls pipelined two half-groups behind ----
        acc = {}
        sqs = {}

        def emit_ssq(h):
            gh = h // 2
            for ml in range(MH):
                mgh = (h % 2) * MH + ml
                c0 = ml * K
                for ch in range(2):
                    nc.tensor.matmul(
                        out=acc[gh][:, P + mgh : P + mgh + 1],
                        lhsT=sqs[h][:, ch, c0 : c0 + P],
                        rhs=ones_mov[:],
                        start=(ch == 0), stop=(ch == 1),
                    )
            if h % 2 == 1:
                accsb = scp.tile([P, 2 * P], f32, tag="accsb")
                nc.vector.tensor_copy(accsb[:], acc[gh][:])
                tp = tpp.tile([P, 2 * P], f32, tag="tp", name=f"tp{gh}")
                nc.tensor.transpose(tp[:, 0:P], accsb[:, 0:P], identity[:])
                nc.tensor.transpose(
                    tp[:, P : 2 * P], accsb[:, P : 2 * P], identity[:]
                )
                # per-group epilogue: 1/|neg| = exp(-0.5*ln(ssq)), logits,
                # exp-accum for logsumexp — all in the same ACT table
                d = gt[gh]
                lns = scrp.tile([P, K], f32, tag="lns")
                nc.scalar.activation(out=lns[:], in_=tp[:, P : P + K], func=Act.Ln)
                nc.scalar.activation(
                    out=nrn_a[:, gh * K : (gh + 1) * K], in_=lns[:],
                    func=Act.Exp, scale=-0.5,
                )
                nc.vector.scalar_tensor_tensor(
                    out=d["logits"][:, 0:1], in0=cpd_a[:, gh : gh + 1],
                    scalar=crn_a[:, gh : gh + 1], in1=prn_a[:, gh : gh + 1],
                    op0=Alu.mult, op1=Alu.mult,
                )
                nc.vector.scalar_tensor_tensor(
                    out=d["logits"][:, 1 : K + 1], in0=tp[:, 0:K],
                    scalar=crn_a[:, gh : gh + 1],
                    in1=nrn_a[:, gh * K : (gh + 1) * K],
                    op0=Alu.mult, op1=Alu.mult,
                )
                esc = scrp.tile([P, K + 1], f32, tag="esc")
                nc.scalar.activation(
                    out=esc[:], in_=d["logits"][:], func=Act.Exp,
                    scale=1.0 / TEMP, accum_out=se_a[:, gh : gh + 1],
                )

        for hg in range(HG):
            g, half = hg // 2, hg % 2
            nt = nts[hg]
            if hg + 2 < HG:
                nts[hg + 2] = ntp.tile(
                    [P, 2, NCOL], fp8, tag="nt", name=f"nt{hg + 2}"
                )
                nc.sync.dma_start(out=nts[hg + 2][:], in_=negt[hg + 2])
            sq = sqp.tile([P, 2, NCOL], fp8, tag="sq")
            sqs[hg] = sq
            with nc.allow_low_precision(reason="fp8 squares feed fp8 matmul"):
                nc.scalar.activation(
                    out=sq[:, :, 0:ACT_SQ], in_=nt[:, :, 0:ACT_SQ],
                    func=Act.Square,
                )
                nc.vector.tensor_tensor(
                    out=sq[:, :, ACT_SQ : ACT_SQ + DVE_SQ],
                    in0=nt[:, :, ACT_SQ : ACT_SQ + DVE_SQ],
                    in1=nt[:, :, ACT_SQ : ACT_SQ + DVE_SQ],
                    op=Alu.mult,
                )
                nc.gpsimd.tensor_tensor(
                    out=sq[:, :, ACT_SQ + DVE_SQ : NCOL],
                    in0=nt[:, :, ACT_SQ + DVE_SQ : NCOL],
                    in1=nt[:, :, ACT_SQ + DVE_SQ : NCOL],
                    op=Alu.mult,
                )
            if half == 0:
                acc[g] = accp.tile([P, 2 * P], f32, tag="acc", name=f"acc{g}")
            for ml in range(MH):
                mg = half * MH + ml
                m = hg * MH + ml
                c0 = ml * K
                for ch in range(2):
                    nc.tensor.matmul(
                        out=acc[g][:, mg : mg + 1],
                        lhsT=nt[:, ch, c0 : c0 + P],
                        rhs=ctxt_s[:, ch, m : m + 1],
                        start=(ch == 0), stop=(ch == 1),
                    )
            if hg == 1:
                emit_prologue()
            if hg >= 2:
                emit_ssq(hg - 2)
        emit_ssq(HG - 2)
        emit_ssq(HG - 1)

        # ---- tail: logsumexp finish ----
        nc.scalar.activation(out=lnse_a[:], in_=se_a[:], func=Act.Ln)
        for g in range(G):
            nc.vector.scalar_tensor_tensor(
                out=out_t[:, g : g + 1], in0=gt[g]["logits"][:, 0:1],
                scalar=-1.0 / TEMP, in1=lnse_a[:, g : g + 1],
                op0=Alu.mult, op1=Alu.add,
            )
        nc.sync.dma_start(out=rowloss[:], in_=out_t[:])
    nc.finalize()
    return nc


def _get_nc():
    global _NC
    if _NC is None:
        _NC = _build_nc()
    return _NC


def make_in_maps(context, positive, negatives, mask_indices):
    import ml_dtypes

    bf = ml_dtypes.bfloat16
    f8 = ml_dtypes.float8_e4m3
    context = np.asarray(context, dtype=np.float32)
    positive = np.asarray(positive, dtype=np.float32)
    negatives = np.asarray(negatives, dtype=np.float32)
    mask = np.asarray(mask_indices).astype(bool)
    in_maps = []
    for b in range(B):
        idx = np.flatnonzero(mask[b])
        assert idx.size == M, f"row {b}: expected {M} masked, got {idx.size}"
        ctx_m = np.ascontiguousarray(context[b].T[idx])  # [M, C] f32
        pos_m = np.ascontiguousarray(positive[b].T[idx])  # [M, C] f32
        neg = negatives[b]  # [M, K, C] f32
        # negT [2, 128, M, K]: c split into (i=c//128, c'=c%128)
        negT = neg.transpose(2, 0, 1).reshape(2, P, M, K).astype(f8)
        nt = np.zeros((HG, P, 2, NCOL), dtype=f8)
        nt[:, :, :, : MH * K] = negT.reshape(2, P, HG, MH * K).transpose(
            2, 1, 0, 3
        )  # [HG, P, 2, NCOL]
        ctxT = ctx_m.T.astype(f8)  # [C, M]
        ctxt = np.ascontiguousarray(
            ctxT.reshape(2, P, M).transpose(1, 0, 2)
        )  # [P, 2, M]
        in_maps.append(
            {
                "negt": nt,
                "ctxt": ctxt,
                "ctxg": ctx_m.astype(bf),
                "posg": pos_m.astype(bf),
            }
        )
    return in_maps


def kernel(context, positive, negatives, mask_indices, num_masked):
    from concourse.bass_utils import run_bass_kernel_spmd

    nm = int(np.asarray(num_masked))
    assert nm == M, f"kernel hardcodes num_masked={M}, got {nm}"
    assert np.asarray(context).shape == (B, C, T)
    assert np.asarray(negatives).shape == (B, M, K, C)

    in_maps = make_in_maps(context, positive, negatives, mask_indices)
    res = run_bass_kernel_spmd(_get_nc(), in_maps, core_ids=list(range(B)))
    total = np.float64(0.0)
    for r in res.results:
        total += r["rowloss"].astype(np.float64).sum()
    return np.float32(total / (B * M))


# revision 11
# speedup vs baseline: 1.0132x; 1.0132x over previous
"""Trainium2 Bass kernel for nn_ContrastiveLoss (wav2vec2-style) — v4.

Shapes (hardcoded): B=8, C=256, T=1024, M=512 masked positions, K=100
negatives. Sharding: pure data parallel — batch row b -> NeuronCore b.

Design: ONE fp8 stream of the negatives (13.1 MB/core) in c-major layout
[HG=8][128, 2, 6400] (dim1 = the two 128-channel halves). Per masked
position m the PE runs four ldweights+matmul pairs with 100-col
stationaries:

- dots^T[k, m]: lhsT = negT[m] chunk [128c, 100k] fp8, rhs = ctx[m] col,
  PSUM-accumulated over the two c-halves.
- ssq^T[k, m]:  lhsT = sq[m] chunk (squared negatives), rhs = ones col.

The elementwise squares are split across ScalarE / VectorE / GpSimd
(fp8 -> fp8, one 2D op per plane per engine per tile, shares sized by
measured rates). The ssq matmuls run one half-group behind the dots so
the PE never stalls waiting for squares. Both PE outputs share one
[128, 256] PSUM tile per group and a rectangular copy + PE-transpose
([100, 128] -> [128, 100]) into per-group [128, 200] PSUM tiles that the
epilogue (norms, logits, exp-accum logsumexp) reads directly.
"""

import numpy as np

TEMP = 0.1
EPS = 1e-8
B, C, T = 8, 256, 1024
M = 512  # masked positions per batch row
K = 100  # negatives per masked position
P = 128  # partitions
G = M // P  # m-groups per core (4)
HG = 8  # half-groups (64 m's each)
MH = M // HG  # 64
NCOL = MH * K + P  # 6528 streamed cols per half-group (incl 128-col zero pad)
# elementwise-square split (cols of each [128, 2, NCOL] tile)
# measured rates: ACT 0.857 ns/col, DVE 1.39 (fp8 1x), Pool 2.35
ACT_SQ = 3530
DVE_SQ = 1830
POOL_SQ = NCOL - ACT_SQ - DVE_SQ  # 1168

_NC = None


def _build_nc():
    import concourse.bacc as bacc
    import concourse.tile as tile
    from concourse import masks, mybir

    f32 = mybir.dt.float32
    bf16 = mybir.dt.bfloat16
    fp8 = mybir.dt.float8e4
    Alu = mybir.AluOpType
    Act = mybir.ActivationFunctionType

    nc = bacc.Bacc(trn_type="TRN2")
    negt = nc.dram_tensor("negt", [HG, P, 2, NCOL], fp8, kind="ExternalInput")
    ctxt = nc.dram_tensor("ctxt", [P, 2, M], fp8, kind="ExternalInput")
    ctxg = nc.dram_tensor("ctxg", [M, C], bf16, kind="ExternalInput")
    posg = nc.dram_tensor("posg", [M, C], bf16, kind="ExternalInput")
    rowloss = nc.dram_tensor("rowloss", [P, G], f32, kind="ExternalOutput")

    from contextlib import ExitStack

    with tile.TileContext(nc) as tc, ExitStack() as es:
        pool_specs = dict(single=1, ntp=4, sqp=4, grp=2, pg=G, scrp=2, scp=2)
        pools = {
            n: es.enter_context(tc.tile_pool(name=n, bufs=b))
            for n, b in pool_specs.items()
        }
        single, ntp, sqp, grp, pg, scrp, scp = (
            pools[n] for n in ("single", "ntp", "sqp", "grp", "pg", "scrp", "scp")
        )
        accp = es.enter_context(tc.psum_pool(name="acc", bufs=4))
        tpp = es.enter_context(tc.psum_pool(name="tp", bufs=2))

        # Pin the one ACT table that serves Square+Ln+Exp+Copy so the
        # compiler's greedy per-function set choice can't ping-pong tables
        # (each implicit ACT_TABLE_LOAD costs 1283ns).
        from concourse.hw_specs import get_activation_tables

        _tabs = list(get_activation_tables(nc.m.arch).items())
        _need = {Act.Square, Act.Ln, Act.Exp, Act.Copy}
        _set_id = next(
            i for i, (_n, _s) in enumerate(_tabs) if _need <= _s
        )
        nc.scalar.add_instruction(
            mybir.InstLoadActFuncSet(
                name=nc.get_next_instruction_name(),
                ins=[], outs=[], act_func_set_id=_set_id,
            )
        )

        identity = single.tile([P, P], f32)
        masks.make_identity(nc, identity[:])
        ones_mov = single.tile([P, 1], fp8, name="ones")
        nc.gpsimd.memset(ones_mov[:], 1.0)
        dummy = single.tile([P, C], bf16)

        nts = {}
        nts[0] = ntp.tile([P, 2, NCOL], fp8, tag="nt", name="nt0")
        nc.sync.dma_start(
            out=nts[0][:, :, 0:ACT_SQ], in_=negt[0, :, :, 0:ACT_SQ]
        )
        nc.sync.dma_start(
            out=nts[0][:, :, ACT_SQ:NCOL], in_=negt[0, :, :, ACT_SQ:NCOL]
        )
        nts[1] = ntp.tile([P, 2, NCOL], fp8, tag="nt", name="nt1")
        nc.sync.dma_start(out=nts[1][:], in_=negt[1])
        ctxt_s = single.tile([P, 2, M], fp8, name="ctxts")
        nc.sync.dma_start(out=ctxt_s[:], in_=ctxt[:, :, :])

        css_a = single.tile([P, G], f32)
        pss_a = single.tile([P, G], f32)
        cpd_a = single.tile([P, G], f32)
        crn_a = single.tile([P, G], f32)
        prn_a = single.tile([P, G], f32)
        se_a = single.tile([P, G], f32)
        lnse_a = single.tile([P, G], f32)
        nrn_a = single.tile([P, G * K], f32)
        out_t = single.tile([P, G], f32)

        gt = {}
        for g in range(G):
            gt[g] = dict(
                logits=pg.tile([P, K + 1], f32, tag="logits", name=f"logits{g}"),
            )

        def emit_prologue():
            # ctx/pos self-norms via ACT Square+accum; ctx.pos dot on DVE;
            # emitted after the first stream tiles so it doesn't delay them
            for g in range(G):
                m0 = g * P
                ctx_t = grp.tile([P, C], bf16, tag="ctx", name=f"ctx{g}")
                pos_t = grp.tile([P, C], bf16, tag="pos", name=f"pos{g}")
                nc.sync.dma_start(out=ctx_t[:], in_=ctxg[m0 : m0 + P, :])
                nc.sync.dma_start(out=pos_t[:], in_=posg[m0 : m0 + P, :])
                nc.scalar.activation(
                    out=dummy[:], in_=ctx_t[:], func=Act.Square,
                    accum_out=css_a[:, g : g + 1],
                )
                nc.scalar.activation(
                    out=dummy[:], in_=pos_t[:], func=Act.Square,
                    accum_out=pss_a[:, g : g + 1],
                )
                nc.vector.scalar_tensor_tensor(
                    out=dummy[:], in0=ctx_t[:], scalar=1.0, in1=pos_t[:],
                    op0=Alu.mult, op1=Alu.mult, accum_out=cpd_a[:, g : g + 1],
                )
            # 1/|ctx|, 1/|pos| = exp(-0.5*ln(ss)); same ACT table as Square
            nc.scalar.activation(out=lnse_a[:], in_=css_a[:], func=Act.Ln)
            nc.scalar.activation(
                out=crn_a[:], in_=lnse_a[:], func=Act.Exp, scale=-0.5
            )
            nc.scalar.activation(out=lnse_a[:], in_=pss_a[:], func=Act.Ln)
            nc.scalar.activation(
                out=prn_a[:], in_=lnse_a[:], func=Act.Exp, scale=-0.5
            )

        # ---- main stream, ssq matmuls pipelined two half-groups behind ----
        acc = {}
        sqs = {}

        def emit_ssq(h):
            gh = h // 2
            for ml in range(MH):
                mgh = (h % 2) * MH + ml
                c0 = ml * K
                for ch in range(2):
                    nc.tensor.matmul(
                        out=acc[gh][:, P + mgh : P + mgh + 1],
                        lhsT=sqs[h][:, ch, c0 : c0 + P],
                        rhs=ones_mov[:],
                        start=(ch == 0), stop=(ch == 1),
                    )
            if h % 2 == 1:
                accsb = scp.tile([P, 2 * P], f32, tag="accsb")
                nc.vector.tensor_copy(accsb[:], acc[gh][:])
                tp = tpp.tile([P, 2 * P], f32, tag="tp", name=f"tp{gh}")
                nc.tensor.transpose(tp[:, 0:P], accsb[:, 0:P], identity[:])
                nc.tensor.transpose(
                    tp[:, P : 2 * P], accsb[:, P : 2 * P], identity[:]
                )
                # per-group epilogue: 1/|neg| = exp(-0.5*ln(ssq)), logits,
                # exp-accum for logsumexp — all in the same ACT table
                d = gt[gh]
                lns = scrp.tile([P, K], f32, tag="lns")
                nc.scalar.activation(out=lns[:], in_=tp[:, P : P + K], func=Act.Ln)
                nc.scalar.activation(
                    out=nrn_a[:, gh * K : (gh + 1) * K], in_=lns[:],
                    func=Act.Exp, scale=-0.5,
                )
                nc.vector.scalar_tensor_tensor(
                    out=d["logits"][:, 0:1], in0=cpd_a[:, gh : gh + 1],
                    scalar=crn_a[:, gh : gh + 1], in1=prn_a[:, gh : gh + 1],
                    op0=Alu.mult, op1=Alu.mult,
                )
                nc.vector.scalar_tensor_tensor(
                    out=d["logits"][:, 1 : K + 1], in0=tp[:, 0:K],
                    scalar=crn_a[:, gh : gh + 1],
                    in1=nrn_a[:, gh * K : (gh + 1) * K],
                    op0=Alu.mult, op1=Alu.mult,
                )
                esc = scrp.tile([P, K + 1], f32, tag="esc")
                nc.scalar.activation(
                    out=esc[:], in_=d["logits"][:], func=Act.Exp,
                    scale=1.0 / TEMP, accum_out=se_a[:, gh : gh + 1],
                )

        for hg in range(HG):
            g, half = hg // 2, hg % 2
            nt = nts[hg]
            if hg + 2 < HG:
                nts[hg + 2] = ntp.tile(
                    [P, 2, NCOL], fp8, tag="nt", name=f"nt{hg + 2}"
                )
                nc.sync.dma_start(out=nts[hg + 2][:], in_=negt[hg + 2])
            sq = sqp.tile([P, 2, NCOL], fp8, tag="sq")
            sqs[hg] = sq
            with nc.allow_low_precision(reason="fp8 squares feed fp8 matmul"):
                nc.scalar.activation(
                    out=sq[:, :, 0:ACT_SQ], in_=nt[:, :, 0:ACT_SQ],
                    func=Act.Square,
                )
                nc.vector.tensor_tensor(
                    out=sq[:, :, ACT_SQ : ACT_SQ + DVE_SQ],
                    in0=nt[:, :, ACT_SQ : ACT_SQ + DVE_SQ],
                    in1=nt[:, :, ACT_SQ : ACT_SQ + DVE_SQ],
                    op=Alu.mult,
                )
                nc.gpsimd.tensor_tensor(
                    out=sq[:, :, ACT_SQ + DVE_SQ : NCOL],
                    in0=nt[:, :, ACT_SQ + DVE_SQ : NCOL],
                    in1=nt[:, :, ACT_SQ + DVE_SQ : NCOL],
                    op=Alu.mult,
                )
            if half == 0:
                acc[g] = accp.tile([P, 2 * P], f32, tag="acc", name=f"acc{g}")
            for ml in range(MH):
                mg = half * MH + ml
                m = hg * MH + ml
                c0 = ml * K
                for ch in range(2):
                    nc.tensor.matmul(
                        out=acc[g][:, mg : mg + 1],
                        lhsT=nt[:, ch, c0 : c0 + P],
                        rhs=ctxt_s[:, ch, m : m + 1],
                        start=(ch == 0), stop=(ch == 1),
                    )
            if hg == 1:
                emit_prologue()
            if hg >= 2:
                emit_ssq(hg - 2)
        emit_ssq(HG - 2)
        emit_ssq(HG - 1)

        # ---- tail: logsumexp finish ----
        nc.scalar.activation(out=lnse_a[:], in_=se_a[:], func=Act.Ln)
        for g in range(G):
            nc.vector.scalar_tensor_tensor(
                out=out_t[:, g : g + 1], in0=gt[g]["logits"][:, 0:1],
                scalar=-1.0 / TEMP, in1=lnse_a[:, g : g + 1],
                op0=Alu.mult, op1=Alu.add,
            )
        nc.sync.dma_start(out=rowloss[:], in_=out_t[:])
    nc.finalize()
    return nc


def _get_nc():
    global _NC
    if _NC is None:
        _NC = _build_nc()
    return _NC


def make_in_maps(context, positive, negatives, mask_indices):
    import ml_dtypes

    bf = ml_dtypes.bfloat16
    f8 = ml_dtypes.float8_e4m3
    context = np.asarray(context, dtype=np.float32)
    positive = np.asarray(positive, dtype=np.float32)
    negatives = np.asarray(negatives, dtype=np.float32)
    mask = np.asarray(mask_indices).astype(bool)
    in_maps = []
    for b in range(B):
        idx = np.flatnonzero(mask[b])
        assert idx.size == M, f"row {b}: expected {M} masked, got {idx.size}"
        ctx_m = np.ascontiguousarray(context[b].T[idx])  # [M, C] f32
        pos_m = np.ascontiguousarray(positive[b].T[idx])  # [M, C] f32
        neg = negatives[b]  # [M, K, C] f32
        # negT [2, 128, M, K]: c split into (i=c//128, c'=c%128)
        negT = neg.transpose(2, 0, 1).reshape(2, P, M, K).astype(f8)
        nt = np.zeros((HG, P, 2, NCOL), dtype=f8)
        nt[:, :, :, : MH * K] = negT.reshape(2, P, HG, MH * K).transpose(
            2, 1, 0, 3
        )  # [HG, P, 2, NCOL]
        ctxT = ctx_m.T.astype(f8)  # [C, M]
        ctxt = np.ascontiguousarray(
            ctxT.reshape(2, P, M).transpose(1, 0, 2)
        )  # [P, 2, M]
        in_maps.append(
            {
                "negt": nt,
                "ctxt": ctxt,
                "ctxg": ctx_m.astype(bf),
                "posg": pos_m.astype(bf),
            }
        )
    return in_maps


def kernel(context, positive, negatives, mask_indices, num_masked):
    from concourse.bass_utils import run_bass_kernel_spmd

    nm = int(np.asarray(num_masked))
    assert nm == M, f"kernel hardcodes num_masked={M}, got {nm}"
    assert np.asarray(context).shape == (B, C, T)
    assert np.asarray(negatives).shape == (B, M, K, C)

    in_maps = make_in_maps(context, positive, negatives, mask_indices)
    res = run_bass_kernel_spmd(_get_nc(), in_maps, core_ids=list(range(B)))
    total = np.float64(0.0)
    for r in res.results:
        total += r["rowloss"].astype(np.float64).sum()
    return np.float32(total / (B * M))
